# revision 34
# baseline (speedup 1.0000x reference)
"""BiLSTM-CRF NLL kernel for 8 TRN2 NeuronCores (v2).

Sharding: data-parallel over batch. B=128 split into 8 shards of 16
sentences; each core runs both LSTM directions, the fc projection, the
CRF forward pass and the gold-path score for its shard.

v2 design (vs baseline):
  - W_ih folded into the embedding table on the host:
    preW[v] = emb[v] @ W_ih^T + (b_ih + b_hh), bf16, rows permuted to
    [i|f|o|g] with the g block pre-scaled by 2 (tanh(g) = 2*sigmoid(2g)-1).
    The per-step input contribution is a single indirect row gather +
    16 PE transposes + one DVE copy per 8-step window per direction.
  - Gates accumulate in PSUM: identity-matmul injects the pre slice,
    then 64 bf16 W_hh matmuls accumulate on top (start=False). No
    separate pre-add on the elementwise chain.
  - Per-step elementwise (per dir): 2 sigmoids (gi, fo views), fused
    tanh(g) via tensor_scalar 2s-1, 4-5 DVE tensor ops, 1 tanh.
    Forward dir runs unmasked (post-length values unused); backward
    keeps masked state in cst/hcurb via copy_predicated.
  - CRF: stationary matrix augmented to [12,13] with an all-ones column
    so every step's matmul also yields the column sum (for renorm)
    for free; renormalization is applied two epochs late off the
    critical chain; per-step constant e^-2.5 damping (compensated by
    +2.5*len at the end) keeps magnitudes in f32 range.
"""

import os
import numpy as np
import ml_dtypes

import concourse.bass as bass
import concourse.bacc as bacc
import concourse.mybir as mybir
import concourse.tile as tile
from concourse.bass import AP
from concourse.masks import make_identity

F32 = mybir.dt.float32
BF16 = mybir.dt.bfloat16
I32 = mybir.dt.int32
U8 = mybir.dt.uint8
MUL = mybir.AluOpType.mult
ADD = mybir.AluOpType.add
SUB = mybir.AluOpType.subtract
X = mybir.AxisListType.X
SIG = mybir.ActivationFunctionType.Sigmoid
TANH = mybir.ActivationFunctionType.Tanh
EXP = mybir.ActivationFunctionType.Exp
LN = mybir.ActivationFunctionType.Ln

P = 128
B = 16            # batch per core
H = 512
G = 2048          # 4H
K = 12
START, STOP = 10, 11
R = 8             # CRF renorm epoch length
W = 8             # pre-gather window (steps per indirect gather)
NCORES = 8
SHIFT = 2.5       # per-step CRF damping exp(-SHIFT)

T = int(os.environ.get("BASS_LSTM_T", "256"))
SKIP = set(os.environ.get("BASS_SKIP", "").split(","))
NW = T // W       # number of gather windows
NJ = T // R - 1   # number of CRF renorm epochs with a recorded sum


def fv(t, off, pat):
    """Free-dim view of a contiguous [P, F] tile: keep partition pair, replace
    free dims with `pat` (list of [step, count]) at element offset `off`."""
    base = t[:] if not isinstance(t, AP) else t
    part = list(base.ap[0])
    return AP(base.tensor, base.offset + off, [part] + [list(p) for p in pat])


def build(nc):
    dirs = ("f", "b")
    dt = {}

    def din(name, shape, dtype):
        dt[name] = nc.dram_tensor(name, shape, dtype, kind="ExternalInput")
        return dt[name]

    for d in dirs:
        din(f"xw_{d}", [T * B], I32)
        din(f"preW_{d}", [30000, G], BF16)
        din(f"whhT_{d}", [H, G], BF16)
        din(f"h0T_{d}", [P, 64], BF16)
        din(f"c0T_{d}", [P, 64], F32)
        din(f"fcWT_{d}", [H, K], BF16)
    din("mask_b", [T, P, 64], U8)
    din("transT", [K, K], F32)
    din("trans", [K, K], F32)
    din("fcb", [K], F32)
    din("a0", [K, B], F32)
    din("msel", [K, (T // 2) * B], F32)
    din("mcomp", [32 * B], F32)
    din("min1", [K, (T // 2) * B], U8)
    din("min2", [K, (T // 2) * B], BF16)
    din("mA", [B], U8)
    din("lenc", [B], F32)
    din("sel", [K, T * B], F32)
    din("counts", [B, 144], F32)
    din("cntb", [B, K], F32)

    nll_o = nc.dram_tensor("nll", [B], F32, kind="ExternalOutput")
    demis_o = nc.dram_tensor("dbg_emis", [K, T * B], F32, kind="ExternalOutput")
    dlogz_o = nc.dram_tensor("dbg_logz", [B], F32, kind="ExternalOutput")
    dgold_o = nc.dram_tensor("dbg_gold", [B], F32, kind="ExternalOutput")
    dhs_o = None
    if os.environ.get("BASS_DBG"):
        dhs_o = {d: nc.dram_tensor(f"dbg_hs_{d}", [P, (T + 1) * 64], BF16,
                                   kind="ExternalOutput") for d in ("f", "b")}
    dg0_o = None
    if os.environ.get("BASS_DBG"):
        dg0_o = nc.dram_tensor("dbg_g0", [P, 512], F32, kind="ExternalOutput")
        dpre_o = nc.dram_tensor("dbg_pre0", [P, G], BF16, kind="ExternalOutput")

    scr16 = nc.dram_tensor("scr16", [B], F32)

    with tile.TileContext(nc) as tc:
        with tc.tile_pool(name="persist", bufs=1) as pp:
            whh = {d: pp.tile([P, 4 * 16 * P], BF16, name=f"whh{d}", tag=f"whh{d}")
                   for d in dirs}
            fcw = {d: pp.tile([P, 4 * K], BF16, name=f"fcw{d}", tag=f"fcw{d}") for d in dirs}
            hs = {d: pp.tile([P, (T + 1) * 64], BF16, name=f"hs{d}", tag=f"hs{d}")
                  for d in dirs}
            cst = {d: pp.tile([P, 64], F32, name=f"cst{d}", tag=f"c{d}") for d in dirs}
            hcurb = pp.tile([P, 64], BF16, tag="hcurb")
            identB = pp.tile([P, P], BF16, tag="identB")
            emisT = pp.tile([K, T * B], F32, tag="emisT")
            hist = pp.tile([K, T * B], F32, tag="hist")
            expem = pp.tile([K, T * B], F32, tag="expem")
            Sall = pp.tile([1, (NJ + 1) * B], F32, tag="Sall")
            idxall = {d: pp.tile([P, NW], I32, name=f"idxall{d}", tag=f"idxall{d}")
                      for d in dirs}

            make_identity(nc, identB[:])
            nc.gpsimd.memset(Sall[:], 1.0)
            for d in dirs:
                for k in range(4):
                    nc.sync.dma_start(
                        whh[d][:, k * 16 * P:(k + 1) * 16 * P],
                        dt[f"whhT_{d}"].ap()[k * P:(k + 1) * P, :])
                    nc.sync.dma_start(
                        fcw[d][:, k * K:(k + 1) * K],
                        dt[f"fcWT_{d}"].ap()[k * P:(k + 1) * P, :])
                nc.sync.dma_start(hs[d][:, 0:64], dt[f"h0T_{d}"].ap()[:])
                nc.sync.dma_start(cst[d][:], dt[f"c0T_{d}"].ap()[:])
                nc.sync.dma_start(
                    idxall[d][:], AP(dt[f"xw_{d}"], 0, [[1, P], [P, NW]]))
            nc.sync.dma_start(hcurb[:], dt["h0T_b"].ap()[:])

            # ---- recurrence with inlined pre-staging ----
            with tc.tile_pool(name="rec_sbuf", bufs=2) as rp, \
                 tc.tile_pool(name="stage_psum", bufs=2, space="PSUM") as stp, \
                 tc.tile_pool(name="gate_psum", bufs=2, space="PSUM") as gpp:

                prechW = {}
                maskch = None

                rowsbuf = {}
                stgbuf = {}

                def stage_gather(w, d):
                    rows = rp.tile([P, G], BF16, name=f"rows{d}", tag=f"rows{d}",
                                   bufs=2)
                    nc.gpsimd.indirect_dma_start(
                        out=rows[:], out_offset=None,
                        in_=dt[f"preW_{d}"].ap()[:],
                        in_offset=bass.IndirectOffsetOnAxis(
                            ap=idxall[d][:, w:w + 1], axis=0))
                    rowsbuf[d] = rows

                def stage_tr(d, half):
                    if half == 0:
                        stgbuf[d] = stp.tile([P, G], BF16, name=f"stg{d}", tag="stg")
                    stg, rows = stgbuf[d], rowsbuf[d]
                    for m in range(8 * half, 8 * half + 8):
                        nc.tensor.transpose(
                            stg[:, m * P:(m + 1) * P], rows[:, m * P:(m + 1) * P],
                            identB[:])

                def stage_copy(d, half=None):
                    # quarter-copies keep the DVE queue blocks short so chain
                    # ops aren't head-of-line blocked behind a window copy
                    if half in (0, None):
                        pc = rp.tile([P, G], BF16, name=f"prech{d}",
                                     tag=f"prech{d}", bufs=3)
                        stage_copy.cur[d] = pc
                    pc = stage_copy.cur[d]
                    if half is None:
                        nc.vector.tensor_copy(pc[:], stgbuf[d][:])
                    else:
                        for q in (0, 1):
                            h = slice((2 * half + q) * (G // 4),
                                      (2 * half + q + 1) * (G // 4))
                            nc.vector.tensor_copy(pc[:, h], stgbuf[d][:, h])
                    return pc
                stage_copy.cur = {}

                def stage(w, d):
                    stage_gather(w, d)
                    stage_tr(d, 0)
                    stage_tr(d, 1)
                    return stage_copy(d)

                def load_mask(w):
                    mk = rp.tile([P, W * 64], U8, tag="maskch")
                    nc.sync.dma_start(
                        mk[:], AP(dt["mask_b"], w * W * P * 64,
                                  [[64, P], [P * 64, W], [1, 64]]))
                    return mk

                if "rec" not in SKIP:
                    for d in dirs:
                        prechW[d] = stage(0, d)
                    nextprech = {d: stage(1, d) for d in dirs}
                    maskch = load_mask(0)
                    nextmask = load_mask(1)

                # gate layout (host-permuted): g=m0..3, i=m4..7, f=m8..11,
                # o=m12..15. g,i matmuls run first so their sigmoid can start
                # while the f,o matmuls still stream; per-m stop flags close
                # each PSUM region as its last k-accumulation lands.
                m_order = list(range(16))

                rec_range = range(0, T) if "rec" not in SKIP else range(0)
                farprech = {}
                for t in rec_range:
                    w, tl = t // W, t % W
                    prep = w + 2 < NW
                    if tl == 0 and prep:
                        for d in dirs:
                            stage_gather(w + 2, d)
                        farmask = load_mask(w + 2)
                    elif tl == 2 and prep:
                        stage_tr("f", 0)
                    elif tl == 3 and prep:
                        stage_tr("f", 1)
                    elif tl == 4 and prep:
                        farprech["f"] = stage_copy("f", 0)
                        stage_tr("b", 0)
                    elif tl == 5 and prep:
                        stage_copy("f", 1)
                        stage_tr("b", 1)
                    elif tl == 6 and prep:
                        farprech["b"] = stage_copy("b", 0)
                    elif tl == 7 and prep:
                        stage_copy("b", 1)
                    psd = {d: gpp.tile([P, 256], F32, name=f"gates{d}",
                                       tag=f"gates{d}") for d in dirs}
                    mkv = maskch[:, tl * 64:(tl + 1) * 64]
                    for d in dirs:
                        nc.tensor.matmul(
                            psd[d][:], identB[:],
                            fv(prechW[d], tl * B, [[P, 16], [1, B]]),
                            start=True, stop=False, skip_group_check=True)
                    for d in dirs:
                        for mi, m in enumerate(m_order):
                            for k in range(4):
                                if d == "f":
                                    rhs = hs[d][:, t * 64 + k * B: t * 64 + (k + 1) * B]
                                else:
                                    rhs = hcurb[:, k * B:(k + 1) * B]
                                nc.tensor.matmul(
                                    psd[d][:, m * B:(m + 1) * B],
                                    whh[d][:, (k * 16 + m) * P:(k * 16 + m + 1) * P],
                                    rhs, start=False,
                                    stop=(k == 3),
                                    skip_group_check=True)
                    # elementwise, cross-direction interleaved so each in-order
                    # engine queue matches expected data-ready times.
                    # views into sfd (bf16): sg=0:64, si=64:128, sf=128:192,
                    # so=192:256.
                    sfd, w2d, t1d, tcd = {}, {}, {}, {}
                    for d in dirs:
                        sfd[d] = rp.tile([P, 256], BF16, name=f"sifo{d}", tag=f"sifo{d}")
                        w2d[d] = rp.tile([P, 64], BF16, name=f"w2{d}", tag=f"w2{d}")
                        t1d[d] = rp.tile([P, 64], F32, name=f"t1{d}", tag=f"t1{d}")
                        tcd[d] = rp.tile([P, 64], BF16, name=f"tc{d}", tag=f"tc{d}")
                    cnb = rp.tile([P, 64], F32, tag="cnb")
                    # single sigmoid per dir (gifo layout: sg=0:64, si=64:128,
                    # sf=128:192, so=192:256). w2 = (sig(2g) - 0.5) * sig(i)
                    nc.scalar.activation(sfd["f"][:], psd["f"][:], SIG)
                    nc.vector.scalar_tensor_tensor(
                        w2d["f"][:], sfd["f"][:, 0:64], 0.5, sfd["f"][:, 64:128],
                        op0=SUB, op1=MUL)
                    nc.gpsimd.tensor_tensor(
                        t1d["f"][:], cst["f"][:], sfd["f"][:, 128:192], op=MUL)
                    nc.vector.scalar_tensor_tensor(
                        cst["f"][:], w2d["f"][:], 2.0, t1d["f"][:], op0=MUL, op1=ADD)
                    nc.scalar.activation(sfd["b"][:], psd["b"][:], SIG)
                    nc.scalar.activation(tcd["f"][:], cst["f"][:], TANH)
                    nc.vector.scalar_tensor_tensor(
                        w2d["b"][:], sfd["b"][:, 0:64], 0.5, sfd["b"][:, 64:128],
                        op0=SUB, op1=MUL)
                    nc.gpsimd.tensor_tensor(
                        t1d["b"][:], cst["b"][:], sfd["b"][:, 128:192], op=MUL)
                    hslot_f = hs["f"][:, (t + 1) * 64:(t + 2) * 64]
                    nc.vector.tensor_tensor(
                        hslot_f, sfd["f"][:, 192:256], tcd["f"][:], op=MUL)
                    nc.vector.scalar_tensor_tensor(
                        cnb[:], w2d["b"][:], 2.0, t1d["b"][:], op0=MUL, op1=ADD)
                    nc.vector.copy_predicated(cst["b"][:], mkv, cnb[:])
                    nc.scalar.activation(tcd["b"][:], cnb[:], TANH)
                    hslot_b = hs["b"][:, (t + 1) * 64:(t + 2) * 64]
                    nc.vector.tensor_tensor(
                        hslot_b, sfd["b"][:, 192:256], tcd["b"][:], op=MUL)
                    nc.vector.copy_predicated(hcurb[:], mkv, hslot_b)
                    if tl == W - 1 and w + 1 < NW:
                        maskch = nextmask
                        prechW = dict(nextprech)
                        if w + 2 < NW:
                            nextmask = farmask
                            nextprech = dict(farprech)

            if dhs_o is not None:
                for d in dirs:
                    nc.sync.dma_start(dhs_o[d].ap()[:], hs[d][:])

            # ---- fc + CRF (interleaved) ----
            with tc.tile_pool(name="crf_sbuf", bufs=2) as cp, \
                 tc.tile_pool(name="crf_persist", bufs=1) as cpr, \
                 tc.tile_pool(name="rs_pool", bufs=3) as rsp, \
                 tc.tile_pool(name="fc_psum", bufs=2, space="PSUM") as fpp, \
                 tc.tile_pool(name="crf_psum", bufs=2, space="PSUM") as cpp:
                # [12, 33] stationary: cols 0:12 = exp(trans)^T, col 32 = ones
                # (colsum lands on out partition 32 — partition reads must be
                # 32-aligned per the BIR verifier).
                etA = cpr.tile([K, 33], F32, tag="etA")
                transTs = cpr.tile([K, K], F32, tag="transTs")
                nc.sync.dma_start(transTs[:], dt["transT"].ap()[:])
                nc.gpsimd.memset(etA[:], 0.0)
                nc.scalar.activation(etA[:, 0:K], transTs[:], EXP)
                nc.gpsimd.memset(etA[:, 32:33], 1.0)
                Estop = cpr.tile([K, 1], F32, tag="Estop")
                nc.scalar.activation(Estop[:], transTs[:, STOP:STOP + 1], EXP)
                fcbm = cpr.tile([K, 1], F32, tag="fcbm")
                nc.sync.dma_start(fcbm[:], AP(dt["fcb"], 0, [[1, K], [1, 1]]))
                nc.vector.tensor_scalar(out=fcbm[:], in0=fcbm[:], scalar1=SHIFT,
                                        scalar2=None, op0=SUB)
                a0 = cpr.tile([K, B], F32, tag="a0")
                nc.sync.dma_start(a0[:], dt["a0"].ap()[:])

                NCH = T * B // 512
                M = T // 2
                # beta-side stationary: exp(trans) (not transposed) + ones col
                EtrA = cpr.tile([K, 33], F32, tag="EtrA")
                transs = cpr.tile([K, K], F32, tag="transs")
                nc.sync.dma_start(transs[:], dt["trans"].ap()[:])
                nc.gpsimd.memset(EtrA[:], 0.0)
                nc.scalar.activation(EtrA[:, 0:K], transs[:], EXP)
                nc.gpsimd.memset(EtrA[:, 32:33], 1.0)
                # beta stores ptt[t] = psc*em1 in hist; the per-sentence
                # end-injection em2 is folded into each consumer matmul:
                # EtrA @ hist_true[t] = EtrA@ptt[t] + EtrA@em2[t].
                # em1 = expem*(1-minit) is built IN-PLACE in expem's upper
                # half (beta never needs the raw values), em2 = expem*
                # Estop_k*minit.
                em2 = cpr.tile([K, M * B], F32, tag="em2")
                min1 = cpr.tile([K, M * B], U8, tag="min1")
                min2 = cpr.tile([K, M * B], BF16, tag="min2")
                nc.sync.dma_start(min1[:], dt["min1"].ap()[:])
                nc.sync.dma_start(min2[:], dt["min2"].ap()[:])
                mselb = cpr.tile([K, M * B], F32, tag="mselb")
                nc.sync.dma_start(mselb[:], dt["msel"].ap()[:])
                selb = cpr.tile([K, T * B], F32, tag="selb")
                nc.sync.dma_start(selb[:], dt["sel"].ap()[:])
                g3p = cpr.tile([K, NCH * B], F32, tag="g3p")
                rsap, rsapB = {}, {}
                psfd = {}

                def fc_mm(c, j, tag):
                    if j == 0:
                        psfd[tag] = fpp.tile([K, 512], F32, tag=f"psf{tag}",
                                             name=f"psf{tag}", bufs=1)
                    psf = psfd[tag]
                    d = dirs[j // 4]
                    k = j % 4
                    if d == "f":
                        rhs = fv(hs[d], (c * 32 + 1) * 64 + k * B,
                                 [[64, 32], [1, B]])
                    else:
                        rhs = fv(hs[d], (T - c * 32) * 64 + k * B,
                                 [[-64, 32], [1, B]])
                    nc.tensor.matmul(psf[:], fcw[d][:, k * K:(k + 1) * K], rhs,
                                     start=(j == 0), stop=(j == 7))

                def fc_fin(c, tag):
                    psf = psfd[tag]
                    sl = slice(c * 512, (c + 1) * 512)
                    nc.scalar.activation(expem[:, sl], psf[:], EXP,
                                         bias=fcbm[:, 0:1])
                    nc.scalar.copy(emisT[:, sl], psf[:])

                def beta_em(c):
                    sl = slice(c * 512, (c + 1) * 512)
                    el = slice((c - 4) * 512, (c - 3) * 512)
                    nc.gpsimd.tensor_tensor(em2[:, el], expem[:, sl],
                                            min2[:, el], op=MUL)
                    nc.gpsimd.tensor_tensor(expem[:, sl], expem[:, sl],
                                            min1[:, el], op=MUL)

                def gold_pre(c):
                    sl = slice(c * 512, (c + 1) * 512)
                    nc.gpsimd.tensor_tensor(selb[:, sl], emisT[:, sl],
                                            selb[:, sl], op=MUL)
                    nc.vector.tensor_reduce(g3p[:, c * B:(c + 1) * B],
                                            fv(selb, c * 512, [[1, B], [B, 32]]),
                                            axis=X, op=ADD)

                aendp = cpr.tile([K, 4 * B], F32, tag="aendp")

                def mselmul(c):
                    # alpha capture: zero all but the t=len-1 column (len<=M);
                    # chunk c's alpha values are final once s passes its range
                    sl = slice(c * 512, (c + 1) * 512)
                    nc.gpsimd.tensor_tensor(hist[:, sl], hist[:, sl],
                                            mselb[:, sl], op=MUL)

                def aend_part(c):
                    nc.vector.tensor_reduce(aendp[:, c * B:(c + 1) * B],
                                            fv(hist, c * 512, [[1, B], [B, 32]]),
                                            axis=X, op=ADD)

                if "crf" not in SKIP:
                    # beta start: ptt[T-1] = 0 (em2[T-1] carries the len==T
                    # injection into the first beta matmul).
                    nc.gpsimd.memset(hist[:, (T - 1) * B:T * B], 0.0)
                    for j in range(8):
                        fc_mm(0, j, "a")
                    fc_fin(0, "a")
                    for j in range(8):
                        fc_mm(NCH - 1, j, "b")
                    fc_fin(NCH - 1, "b")
                    beta_em(NCH - 1)

                # forward alpha chain over t=0..M-1 and backward beta chain
                # over t=T-1..M run concurrently, meeting at t=M-1. fc is
                # spread one matmul per step to hide in the chain's PE gaps.
                for s in range(0 if "crf" not in SKIP else M, M):
                    cblk, jf = divmod(s, 32)
                    ca, cb = cblk + 1, NCH - 2 - cblk
                    if cblk < 3:
                        if jf < 8:
                            fc_mm(ca, jf, "a")
                        elif jf == 8:
                            fc_fin(ca, "a")
                        elif jf == 9:
                            gold_pre(ca)
                        elif 16 <= jf < 24:
                            fc_mm(cb, jf - 16, "b")
                        elif jf == 24:
                            fc_fin(cb, "b")
                        elif jf == 25:
                            beta_em(cb)
                        elif jf == 26:
                            gold_pre(cb)
                    if cblk == 0:
                        if jf == 10:
                            gold_pre(0)
                        elif jf == 11:
                            gold_pre(NCH - 1)
                    elif jf == 13:
                        mselmul(cblk - 1)
                    elif jf == 14:
                        aend_part(cblk - 1)
                    doS = (s % R == 0 and s >= R)
                    j = s // R - 1
                    # renorm: prescale the emission column read at this step
                    # (off the matmul->mult chain; equivalent to the old
                    # post-mult apply since em2-injected values stay unscaled)
                    if doS and s >= 2 * R:
                        nc.vector.tensor_tensor(
                            expem[:, s * B:(s + 1) * B],
                            expem[:, s * B:(s + 1) * B],
                            rsap[j - 1][:], op=MUL)
                        tb_app = T - 1 - s
                        nc.vector.tensor_tensor(
                            expem[:, tb_app * B:(tb_app + 1) * B],
                            expem[:, tb_app * B:(tb_app + 1) * B],
                            rsapB[j - 1][:], op=MUL)
                    # alpha step t = s
                    t = s
                    psc = cpp.tile([33, B], F32, tag="psca", name="psca")
                    rhs = a0[:] if t == 0 else hist[:, (t - 1) * B:t * B]
                    nc.tensor.matmul(psc[:], etA[:], rhs, start=True, stop=True)
                    if doS:
                        nc.vector.tensor_copy(Sall[:, j * B:(j + 1) * B],
                                              psc[32:33, :])
                    nc.vector.tensor_tensor(
                        hist[:, t * B:(t + 1) * B], psc[0:K, :],
                        expem[:, t * B:(t + 1) * B], op=MUL)
                    # beta step t = T-1-s
                    tb = T - 1 - s
                    if s >= 1:
                        pscb = cpp.tile([33, B], F32, tag="pscb", name="pscb")
                        nc.tensor.matmul(
                            pscb[:], EtrA[:], hist[:, (tb + 1) * B:(tb + 2) * B],
                            start=True, stop=False, skip_group_check=True)
                        nc.tensor.matmul(
                            pscb[:], EtrA[:],
                            em2[:, (tb + 1 - M) * B:(tb + 2 - M) * B],
                            start=False, stop=True, skip_group_check=True)
                        if doS:
                            nc.vector.tensor_copy(
                                Sall[:, (16 + j) * B:(17 + j) * B],
                                pscb[32:33, :])
                        nc.vector.tensor_tensor(
                            hist[:, tb * B:(tb + 1) * B], pscb[0:K, :],
                            expem[:, tb * B:(tb + 1) * B], op=MUL)
                    if doS:
                        rs1 = cp.tile([1, B], F32, tag="rs1")
                        nc.vector.reciprocal(rs1[:], Sall[:, j * B:(j + 1) * B])
                        ra = rsp.tile([K, B], F32, tag="rsap")
                        nc.gpsimd.partition_broadcast(ra[:], rs1[:])
                        rsap[j] = ra
                        rs2 = cp.tile([1, B], F32, tag="rs2")
                        nc.vector.reciprocal(rs2[:],
                                             Sall[:, (16 + j) * B:(17 + j) * B])
                        rb = rsp.tile([K, B], F32, name="rb", tag="rsapB")
                        nc.gpsimd.partition_broadcast(rb[:], rs2[:])
                        rsapB[j] = rb

                if "crf" not in SKIP:
                    M = T // 2
                    # meet: betam = EtrA @ hist_true[M] (ptt[M] + em2[M])
                    pscm = cpp.tile([33, B], F32, tag="fin", bufs=1)
                    nc.tensor.matmul(pscm[:], EtrA[:],
                                     hist[:, M * B:(M + 1) * B],
                                     start=True, stop=False,
                                     skip_group_check=True)
                    nc.tensor.matmul(pscm[:], EtrA[:], em2[:, 0:B],
                                     start=False, stop=True,
                                     skip_group_check=True)
                    betam = cp.tile([K, B], F32, tag="betam")
                    nc.vector.tensor_copy(betam[:], pscm[0:K, :])
                    pz = cp.tile([K, B], F32, tag="pz")
                    nc.vector.tensor_tensor(pz[:], hist[:, (M - 1) * B:M * B],
                                            betam[:], op=MUL)
                    pszC = cpp.tile([33, B], F32, tag="fin", bufs=1)
                    nc.tensor.matmul(pszC[:], etA[:], pz[:], start=True, stop=True)
                    logzC = cp.tile([1, B], F32, tag="logzC")
                    nc.scalar.activation(logzC[:], pszC[32:33, :], LN)
                    # alpha capture at t = len-1 (len <= M sentences):
                    # chunks 0-2 were folded into the loop; finish chunk 3.
                    mselmul(3)
                    aend_part(3)
                    aend = cp.tile([K, B], F32, tag="aend")
                    nc.vector.tensor_reduce(aend[:], fv(aendp, 0, [[1, B], [B, 4]]),
                                            axis=X, op=ADD)
                    azs = cp.tile([K, B], F32, tag="azs")
                    nc.vector.tensor_scalar(out=azs[:], in0=aend[:],
                                            scalar1=Estop[:, 0:1], scalar2=None,
                                            op0=MUL)
                    psz = cpp.tile([33, B], F32, tag="fin", bufs=1)
                    nc.tensor.matmul(psz[:], etA[:], azs[:],
                                     start=True, stop=True)
                    logz0 = cp.tile([1, B], F32, tag="logz0")
                    nc.scalar.activation(logz0[:], psz[32:33, :], LN)
                    # select alpha/combine, then renorm + damping compensation
                    mab = cp.tile([1, B], U8, tag="mab")
                    nc.sync.dma_start(mab[:], AP(dt["mA"], 0, [[1, 1], [1, B]]))
                    lnS = cp.tile([1, 32 * B], F32, tag="lnS")
                    nc.scalar.activation(lnS[:], Sall[:, 0:32 * B], LN)
                    mpf = cp.tile([1, 32 * B], F32, tag="mpf")
                    nc.sync.dma_start(mpf[:], AP(dt["mcomp"], 0,
                                                   [[1, 1], [1, 32 * B]]))
                    nc.vector.tensor_tensor(lnS[:], lnS[:], mpf[:], op=MUL)
                    Lend = cp.tile([1, B], F32, tag="Lend")
                    nc.vector.tensor_reduce(Lend[:], fv(lnS, 0, [[1, B], [B, 32]]),
                                            axis=X, op=ADD)
                    lencs = cp.tile([1, B], F32, tag="lencs")
                    nc.sync.dma_start(lencs[:], AP(dt["lenc"], 0, [[1, 1], [1, B]]))
                    logzf = cp.tile([1, B], F32, tag="logzf")
                    nc.vector.tensor_copy(logzf[:], logzC[:])
                    nc.vector.copy_predicated(logzf[:], mab[:], logz0[:])
                    nc.vector.tensor_tensor(logzf[:], logzf[:], Lend[:], op=ADD)
                    nc.vector.tensor_tensor(logzf[:], logzf[:], lencs[:], op=ADD)
                    nc.sync.dma_start(AP(dlogz_o, 0, [[1, 1], [1, B]]), logzf[:])
                    nc.sync.dma_start(demis_o.ap()[:], emisT[:])

                    # ---- gold score ----
                    tfl = cp.tile([1, 144], F32, tag="tfl")
                    nc.sync.dma_start(tfl[:], AP(dt["trans"], 0, [[1, 1], [1, 144]]))
                    tfb = cp.tile([B, 144], F32, tag="tfb")
                    nc.gpsimd.partition_broadcast(tfb[:], tfl[:])
                    cnts = cp.tile([B, 144], F32, tag="cnts")
                    nc.sync.dma_start(cnts[:], dt["counts"].ap()[:])
                    pr1 = cp.tile([B, 144], F32, tag="pr1")
                    nc.vector.tensor_tensor(pr1[:], cnts[:], tfb[:], op=MUL)
                    g1 = cp.tile([B, 1], F32, tag="g1")
                    nc.vector.tensor_reduce(g1[:], pr1[:], axis=X, op=ADD)
                    fcbr = cp.tile([1, K], F32, tag="fcbr")
                    nc.sync.dma_start(fcbr[:], AP(dt["fcb"], 0, [[1, 1], [1, K]]))
                    fcbb = cp.tile([B, K], F32, tag="fcbb")
                    nc.gpsimd.partition_broadcast(fcbb[:], fcbr[:])
                    cntbs = cp.tile([B, K], F32, tag="cntbs")
                    nc.sync.dma_start(cntbs[:], dt["cntb"].ap()[:])
                    pr2 = cp.tile([B, K], F32, tag="pr2")
                    nc.vector.tensor_tensor(pr2[:], cntbs[:], fcbb[:], op=MUL)
                    g2 = cp.tile([B, 1], F32, tag="g2")
                    nc.vector.tensor_reduce(g2[:], pr2[:], axis=X, op=ADD)
                    g12 = cp.tile([B, 1], F32, tag="g12")
                    nc.vector.tensor_tensor(g12[:], g1[:], g2[:], op=ADD)
                    nc.sync.dma_start(AP(scr16, 0, [[1, B], [1, 1]]), g12[:])
                    g12r = cp.tile([1, B], F32, tag="g12r")
                    nc.sync.dma_start(g12r[:], AP(scr16, 0, [[1, 1], [1, B]]))

                    g3 = cp.tile([K, B], F32, tag="g3")
                    nc.vector.tensor_reduce(g3[:], fv(g3p, 0, [[1, B], [B, NCH]]),
                                            axis=X, op=ADD)
                    psg = cpp.tile([33, B], F32, tag="psg", bufs=1)
                    nc.tensor.matmul(psg[:], etA[:], g3[:],
                                     start=True, stop=True)
                    goldT = cp.tile([1, B], F32, tag="goldT")
                    nc.vector.tensor_tensor(goldT[:], g12r[:], psg[32:33, :], op=ADD)
                    nc.sync.dma_start(AP(dgold_o, 0, [[1, 1], [1, B]]), goldT[:])
                    nllT = cp.tile([1, B], F32, tag="nllT")
                    nc.vector.tensor_tensor(nllT[:], logzf[:], goldT[:], op=SUB)
                    nc.sync.dma_start(AP(nll_o, 0, [[1, 1], [1, B]]), nllT[:])
    return nc


_CACHE = {}


def get_program():
    if "nc" not in _CACHE:
        nc = bacc.Bacc("TRN2", target_bir_lowering=False, debug=False,
                       num_devices=NCORES)
        build(nc)
        nc.compile()
        _CACHE["nc"] = nc
    return _CACHE["nc"]


def perm_gifo(w):
    # [4H, ...] rows i,f,g,o -> g,i,f,o
    return np.concatenate([w[1024:1536], w[0:512], w[512:1024], w[1536:2048]], 0)


def host_prep(inputs):
    f32 = np.float32
    bf = ml_dtypes.bfloat16
    x = np.asarray(inputs["x"]).astype(np.int32)
    lengths = np.asarray(inputs["lengths"]).astype(np.int64)
    tags = np.asarray(inputs["tags"]).astype(np.int64)
    emb = np.asarray(inputs["embedding"], f32)
    trans = np.asarray(inputs["trans"], f32)
    fcW = np.asarray(inputs["fc_W"], f32)
    fcb = np.asarray(inputs["fc_b"], f32)
    h0 = np.asarray(inputs["h0"], f32)
    c0 = np.asarray(inputs["c0"], f32)

    preWd, whhTd = {}, {}
    for d in ("f", "b"):
        wih = perm_gifo(np.asarray(inputs[f"W_ih_{d}"], f32))
        whh = perm_gifo(np.asarray(inputs[f"W_hh_{d}"], f32))
        bi = perm_gifo(np.asarray(inputs[f"b_ih_{d}"], f32)[:, None])[:, 0]
        bh = perm_gifo(np.asarray(inputs[f"b_hh_{d}"], f32)[:, None])[:, 0]
        preW = emb @ wih.T + (bi + bh)[None, :]
        preW[:, 0:512] *= 2.0
        preWd[d] = preW.astype(bf)
        whhT = whh.T.copy()
        whhT[:, 0:512] *= 2.0
        whhTd[d] = whhT.astype(bf).copy()

    fcWT = {"f": fcW[:, :H].T.astype(bf).copy(), "b": fcW[:, H:].T.astype(bf).copy()}

    maps = []
    for c in range(NCORES):
        bs = slice(c * B, (c + 1) * B)
        xs = x[bs]            # [16, T]
        ln = lengths[bs]      # [16]
        tg = tags[bs]         # [16, T]
        m = {"trans": trans, "transT": trans.T.astype(f32).copy(), "fcb": fcb}
        for d in ("f", "b"):
            xt = xs.T if d == "f" else xs.T[::-1]      # [T, 16]
            m[f"xw_{d}"] = np.ascontiguousarray(xt).reshape(-1).astype(np.int32)
            m[f"preW_{d}"] = preWd[d]
            m[f"whhT_{d}"] = whhTd[d]
            m[f"fcWT_{d}"] = fcWT[d]
            di = 0 if d == "f" else 1
            h0T = h0[di, bs].T.reshape(4, P, B).transpose(1, 0, 2).reshape(P, 64)
            c0T = c0[di, bs].T.reshape(4, P, B).transpose(1, 0, 2).reshape(P, 64)
            m[f"h0T_{d}"] = h0T.astype(bf).copy()
            m[f"c0T_{d}"] = c0T.astype(f32).copy()
        # bwd mask: step s processes tau = T-1-s; valid iff tau < len
        tau = (T - 1 - np.arange(T))[:, None]          # [T, 1]
        mk = (tau < ln[None, :]).astype(f32)           # [T, 16]
        m["mask_b"] = np.broadcast_to(
            mk[:, None, None, :], (T, P, 4, B)).reshape(T, P, 64).astype(np.uint8).copy()
        a0 = np.zeros((K, B), f32); a0[START, :] = 1.0
        m["a0"] = a0
        M = T // 2
        msel = np.zeros((K, M, B), f32)
        for b in range(B):
            if ln[b] <= M:
                msel[:, ln[b] - 1, b] = 1.0
        m["msel"] = msel.reshape(K, M * B)
        m["mA"] = (ln <= M).astype(np.uint8)
        # minit[t-M, b]: beta chain injects Estop*u at t = len-1 (len > M).
        # min1 = (1-minit) (keeps the recursive part), min2 = Estop_k*minit
        # (the injected end value, folded into the consumer matmuls).
        minit = np.zeros((M, B), f32)
        for b in range(B):
            if ln[b] > M:
                minit[ln[b] - 1 - M, b] = 1.0
        estop_k = np.exp(trans[STOP, :]).astype(f32)  # [K]
        m["min1"] = np.broadcast_to((1.0 - minit)[None, :, :],
                                    (K, M, B)).reshape(K, M * B).astype(
                                        np.uint8).copy()
        min2 = estop_k[:, None, None] * minit[None, :, :]  # [K, M, B]
        # t = T-1 beta start injects Estop for ALL sentences (keeps the
        # chain nonzero for len<=M columns so ln(Sall) stays finite; those
        # columns' beta values are unused garbage, as in the masked version)
        min2[:, M - 1, :] = estop_k[:, None]
        m["min2"] = min2.reshape(K, M * B).astype(bf)
        # mcomp: slots j<16 = alpha renorms (applied at alpha-step 8j+16),
        # slots 16+j = beta renorms (applied at beta-step 8j+16, counted only
        # after the sentence's injection step s_inj = T - len).
        mcomp = np.zeros((32, B), f32)
        for jj in range(16):
            s_app = R * jj + 2 * R
            cap = np.where(ln <= M, ln - 1, M - 1)
            mcomp[jj] = (s_app <= cap).astype(f32)
            mcomp[16 + jj] = ((ln > M) & (s_app > T - ln) &
                             (s_app <= M - 1)).astype(f32)
        m["mcomp"] = mcomp.reshape(-1)
        m["lenc"] = (SHIFT * ln).astype(f32)
        tarange = np.arange(T)[None, :]
        valid = tarange < ln[:, None]                  # [16, T]
        selm = np.zeros((K, T, B), f32)
        jk = np.arange(K)[:, None, None]
        selm[:] = (tg.T[None] == jk) & valid.T[None]
        m["sel"] = np.ascontiguousarray(selm.reshape(K, T * B))
        counts = np.zeros((B, 144), f32)
        cntb = np.zeros((B, K), f32)
        for b in range(B):
            L = int(ln[b])
            prev = START
            for t in range(L):
                nx = int(tg[b, t])
                counts[b, nx * K + prev] += 1
                cntb[b, nx] += 1
                prev = nx
            counts[b, STOP * K + prev] += 1
        m["counts"] = counts
        m["cntb"] = cntb
        maps.append(m)
    return maps


def kernel(**inputs):
    from concourse.bass_utils import run_bass_kernel_spmd
    nc = get_program()
    maps = host_prep(inputs)
    res = run_bass_kernel_spmd(nc, maps, core_ids=list(range(NCORES)))
    out = np.concatenate([r["nll"] for r in res.results]).astype(np.float32)
    kernel.last_results = res
    return out

